# revision 30
# baseline (speedup 1.0000x reference)
"""Trainium2 Bass kernel for nn_Eq1to3 (eset_ops_1_to_3 + einsum broadcast expansion).

Reference computation (N=16, D=64, S=32, M=48, BASIS=4):
    t[b,n,s,m] = sum_d coefs[d,s,b] * x[n,d,m]        # tiny einsum
    out[n,s,i,j,k] = t0[n,s,i] + t1[n,s,j] + t2[n,s,k]
                     + (i==j==k) * t3[n,s,i] + bias[s]
Output (16, 32, 48, 48, 48) f32 = 226.5 MB. The correctness gate is
scale-relative 2e-2, so the HW computes in bf16 (inputs rounded to bf16,
fp32 PSUM accumulation) and writes bf16 (113 MB total, 14.16 MB per core
=> ~40 us HBM floor at ~358 GB/s); the host upcasts to f32.

Strategy: data-parallel over N across 8 cores (2 batches/core). Per core the
output is [3072 rows p=(n,s,i), 2304 cols (j,k)] in bf16.

Row->partition assignment: partition q = l*64 + ns holds rows
i(q,r) = 24*l + r of (n,s)-group ns = q % 64, l = q // 64, r in [0,24).
The l-halves are CONTIGUOUS partition ranges, so the superdiagonal add
needs no mask tensor: for each half, the diagonal of row u sits at flat
offset u*2304 + i*49 with i = 24*l + r0 + u, i.e. one strided AP of
stride 2353. Every per-partition table is a gather the TENSOR ENGINE
produces directly with host-prepared indicator weights, folding the batch
index into the contraction: lhsT[(n',d), q] = coefs[d, s(q), b]*(n'==n(q)),
against rhs built from x2[(n,d), m] = x[n,d,m] (bf16 matmuls, fp32 PSUM):

    B_ps[q, (r,j)]  = t0[ns(q), i(q,r)] + t1[ns(q), j]
    T2_ps[q, k]     = t2[ns(q), k]
    T3_ps[q, r]     = t3[ns(q), i(q,r)]

ACT mirrors each PSUM B chunk into SBUF bf16 with every value DUPLICATED in
adjacent pairs, B2[q, (r, j, 2)], and T2 (+bias via the ACT per-partition
bias operand) is expanded once into T2E[q, (j, k)] = t2[k] + bias[s] by a
DVE tensor_copy (packed APs -> 4x perf mode). The main loop runs one DVE
tensor_tensor per output row with loop dims (j, 24, 2):

    out[q, (j, t, p)] = B2[q, (r, j, p)](t-bcast) + T2E[q, (j, t, p)]

All three APs are 2-byte, SBUF-resident, with a packed innermost [1,2] dim,
which makes the op eligible for the DVE 2x 16-bit perf mode (~1.35 us/row;
a stride-0 innermost broadcast would pin it at 1x). The t3 superdiagonal
lands as two tiny strided DVE adds per group (one per l-half) read straight
from T3G, so gpsimd carries no compute at all. Output groups stream over
three DMA paths - SP HWDGE, ACT HWDGE (5.31 MB each) and gpsimd SWDGE
(3.54 MB, mid-stream groups) - with 4 single-row groups first (early
stream start), 8 two-row groups, then 4 single-row groups, the last two on
the HWDGE rings so everything drains together. Startup is gated by the
weight load, so w_all is split into two DMAs with the t2/t3 blocks first
(they gate T2E and the first diag), and B2 chunk copies are interleaved
into the ACT queue behind DMA triggers they cannot block.
"""

import numpy as np
import ml_dtypes

N, D, S, M, BASIS = 16, 64, 32, 48, 4
N_CORES = 8
NL = N // N_CORES              # batches per core (2)
NS = NL * S                    # (n,s) groups per core (64)
ROWS = NS * M                  # output rows per core (3072)
JK = M * M                     # free size per row (2304)
P = 128                        # partitions
HALF = M // 2                  # rows per partition (24)
# B-matmul chunk sizes in i'-rows; small first chunks let the HBM write
# stream start before the rest of B exists
CHUNKS = [2, 2, 4, 8, 8]
CHUNK_R0 = [sum(CHUNKS[:c]) for c in range(len(CHUNKS))]
# w_all column blocks (order chosen so the first DMA part carries what
# gates the start of the stream: t2 -> T2E, t3 -> the first diag adds);
# bias is folded into T2G via the ACT per-partition bias operand
W_T2, W_T3L0, W_T3L1, W_T1, W_T0L0, W_T0L1 = range(6)
W_PARTS = [3, 3]               # DMA split: blocks [0,3) then [3,6)

_PROG = None


def _build_prog():
    import concourse.bacc as bacc
    import concourse.tile as tile
    import concourse.mybir as mybir

    bf16 = mybir.dt.bfloat16
    copy_fn = mybir.ActivationFunctionType.Copy
    nc = bacc.Bacc("TRN2", target_bir_lowering=False, debug=False,
                   num_devices=N_CORES)

    w_all_d = nc.dram_tensor("w_all", [NL * D, 6 * P], bf16,
                             kind="ExternalInput").ap()
    bias_d = nc.dram_tensor("bias_q", [P, 1], mybir.dt.float32,
                            kind="ExternalInput").ap()
    x2_d = nc.dram_tensor("x2", [NL * D, M], bf16, kind="ExternalInput").ap()
    y_d = nc.dram_tensor("y", [ROWS, JK], bf16, kind="ExternalOutput").ap()

    K = NL * D                  # contraction size (128)

    with tile.TileContext(nc) as tc:
        with (
            tc.tile_pool(name="const", bufs=1) as cpool,
            tc.tile_pool(name="psum", bufs=1, space="PSUM") as ppool,
            tc.tile_pool(name="outp", bufs=6) as opool,
        ):
            # ---- load inputs; sync ring: x2 + bias, scalar ring: w parts
            x2_sb = cpool.tile([K, M], bf16)
            nc.sync.dma_start(out=x2_sb[:], in_=x2_d[:])
            bias_sb = cpool.tile([P, 1], mybir.dt.float32)
            nc.sync.dma_start(out=bias_sb[:], in_=bias_d[:])
            w_sb = cpool.tile([K, 6 * P], bf16)
            lo = 0
            for nblk in W_PARTS:
                nc.scalar.dma_start(out=w_sb[:, lo * P:(lo + nblk) * P],
                                    in_=w_all_d[:, lo * P:(lo + nblk) * P])
                lo += nblk

            def w_blk(idx):
                return w_sb[:, idx * P:(idx + 1) * P]

            # ---- T2 path first: it gates T2E which gates every row op ----
            T2_ps = ppool.tile([P, M], mybir.dt.float32)
            nc.tensor.matmul(T2_ps[:], w_blk(W_T2), x2_sb[:],
                             start=True, stop=True)
            # ---- T3: gates the diag adds ----
            T3_ps = ppool.tile([P, HALF], mybir.dt.float32)
            for li in range(2):
                nc.tensor.matmul(T3_ps[:], w_blk(W_T3L0 + li),
                                 x2_sb[:, HALF * li:HALF * (li + 1)],
                                 start=(li == 0), stop=(li == 1))

            # bias[s(q)] rides in here: every output element sees T2E once
            T2G = cpool.tile([P, M], bf16)
            nc.scalar.activation(T2G[:], T2_ps[:],
                                 mybir.ActivationFunctionType.Identity,
                                 bias=bias_sb[:, 0:1])
            T3G = cpool.tile([P, HALF], bf16)
            nc.scalar.activation(T3G[:], T3_ps[:], copy_fn)
            HP = P // 2                # partitions per l-half (64)

            def build_t2e(with_diag_of_row):
                # T2E[q, (j, k)] = t2[k]: DVE tensor_copy, all APs packed
                # bf16 SBUF (4x perf mode) - the j-broadcast is an outer
                # dim. Optionally folds row r's t3 diag element in, making
                # that row a single DVE op with no post-diag pass.
                t = cpool.tile([P, JK], bf16)
                nc.vector.tensor_copy(
                    out=t.rearrange("q (j k) -> q j k", k=M),
                    in_=T2G[:, None, :].broadcast_to((P, M, M)))
                if with_diag_of_row is not None:
                    r = with_diag_of_row
                    for l in range(2):
                        s0 = (HALF * l + r) * (M + 1)
                        dv = t[l * HP:(l + 1) * HP, s0:s0 + 1]
                        tg = T3G[l * HP:(l + 1) * HP, r:r + 1]
                        nc.vector.tensor_add(out=dv, in0=dv, in1=tg)
                return t

            # ---- B[q, (r, j)] via accumulating bf16 matmuls, one
            # bank-aligned PSUM tile per chunk (a PE-write and an ACT-read
            # in the same PSUM bank is a hardware fault) ----
            B_chunks = [ppool.tile([P, ci * M], mybir.dt.float32,
                                   name=f"B_ps{c}")
                        for c, ci in enumerate(CHUNKS)]

            def emit_b_chunk(c):
                ci = CHUNKS[c]
                i0 = CHUNK_R0[c]
                blk = B_chunks[c].rearrange("q (r j) -> q r j", j=M)
                # t1 part: rhs[(n'd), (r, j)] = x[n', d, j]
                rhs = x2_sb[:, None, :].broadcast_to((K, ci, M))
                nc.tensor.matmul(blk, w_blk(W_T1), rhs,
                                 start=True, stop=False)
                for li in range(2):
                    # t0 part: rhs[(n'd), (r, j)] = x[n', d, 24*li + i0 + r]
                    rhs = x2_sb[:, HALF * li + i0:HALF * li + i0 + ci]
                    rhs = rhs[:, :, None].broadcast_to((K, ci, M))
                    nc.tensor.matmul(blk, w_blk(W_T0L0 + li), rhs,
                                     start=False, stop=(li == 1))

            # SBUF bf16 mirror of B with values duplicated in pairs:
            # B2[q, (r, j, p)] = B[q, (r, j)] for p=0,1
            B2 = cpool.tile([P, HALF * M * 2], bf16)
            B2v = B2.rearrange("q (r j p) -> q r j p", j=M, p=2)

            def copy_b2(c):
                ci = CHUNKS[c]
                i0 = CHUNK_R0[c]
                src = B_chunks[c].rearrange("q (r j) -> q r j", j=M)
                src = src[:, :, :, None].broadcast_to((P, ci, M, 2))
                nc.scalar.activation(B2v[:, i0:i0 + ci], src, copy_fn)

            # ---- main loop over row-slices r: y row p = r*128 + q, so a
            # group DMA writes one dense contiguous HBM region (partition
            # stride = one 4608 B row) instead of 128 scattered blocks ----
            y_v = y_d.rearrange("(r q) f -> q r f", r=HALF)

            def emit_group(r0, rw, ring, t2e, post_diag=True):
                out_t = opool.tile([P, rw * JK], bf16, tag="out")
                o5 = out_t.rearrange("q (u j t p) -> q u j t p",
                                     u=rw, j=M, p=2)
                t2e5 = t2e.rearrange("q (j t p) -> q j t p", j=M, p=2)
                for u in range(rw):
                    in0 = B2v[:, r0 + u, :, None, :]
                    in0 = in0.broadcast_to((P, M, HALF, 2))
                    nc.vector.tensor_add(out=o5[:, u], in0=in0, in1=t2e5)
                # superdiagonal of row u at flat offset u*2304 + i*49,
                # i = 24*l + r0 + u with l = q//64: stride-2353 AP over
                # each contiguous partition half, straight from T3G - two
                # tiny DVE adds, no mask tensor (the host un-permutes the
                # l-major row order during the gather)
                if post_diag:
                    for l in range(2):
                        half = out_t[l * HP:(l + 1) * HP]
                        s0 = (HALF * l + r0) * (M + 1)
                        dv = half[:, s0::JK + M + 1][:, :rw]
                        tg = T3G[l * HP:(l + 1) * HP, r0:r0 + rw]
                        nc.vector.tensor_add(out=dv, in0=dv, in1=tg)
                ring.dma_start(
                    out=y_v[:, r0:r0 + rw, :],
                    in_=out_t.rearrange("q (u f) -> q u f", u=rw))

            # Row 0 streams via T2E0 (diag pre-folded, built in DVE's idle
            # startup window) so the first DMA fires straight after one
            # row op; everything else uses the shared T2E. Later B2 copies
            # are interleaved so they sit behind DMA triggers they cannot
            # delay. Output rings: SP HWDGE, ACT HWDGE and gpsimd SWDGE
            # carry 4.72 MB each; the last singles end on the two HWDGE
            # rings so everything drains together.
            S, A, G = nc.sync, nc.scalar, nc.gpsimd
            T2E0 = build_t2e(with_diag_of_row=0)
            emit_b_chunk(0)
            copy_b2(0)
            emit_group(0, 1, S, T2E0, post_diag=False)
            T2E = build_t2e(with_diag_of_row=None)
            emit_b_chunk(1)
            copy_b2(1)
            emit_group(1, 1, A, T2E)
            emit_b_chunk(2)
            copy_b2(2)
            emit_group(2, 2, G, T2E)
            emit_b_chunk(3)
            emit_group(4, 2, S, T2E)
            emit_group(6, 2, A, T2E)
            copy_b2(3)
            emit_b_chunk(4)
            emit_group(8, 4, S, T2E)
            copy_b2(4)
            emit_group(12, 4, A, T2E)
            emit_group(16, 4, G, T2E)
            emit_group(20, 2, G, T2E)
            emit_group(22, 1, S, T2E)
            emit_group(23, 1, A, T2E)

    nc.compile()
    return nc


def _get_prog():
    global _PROG
    if _PROG is None:
        _PROG = _build_prog()
    return _PROG


def _make_in_maps(x, coefs, bias):
    x = np.asarray(x, dtype=np.float32)
    coefs = np.asarray(coefs, dtype=np.float32)
    bias = np.asarray(bias, dtype=np.float32)

    # partition q = l*64 + ns: ns(q) = q % 64 = n*32 + s, l(q) = q // 64
    q = np.arange(P)
    ns_of = q % NS
    n_of = ns_of // S
    s_of = ns_of % S
    l_of = q // NS
    # indicator weights w_b[(n',d), q] = coefs[d, s(q), b] * (n' == n(q))
    nd_n = np.repeat(np.arange(NL), D)                # (K,) n' of row
    nd_d = np.tile(np.arange(D), NL)                  # (K,) d of row
    sel = (nd_n[:, None] == n_of[None, :]).astype(np.float32)  # (K, P)

    def w_of(b):
        return coefs[nd_d[:, None], s_of[None, :], b] * sel

    # column blocks: 0=t2, 1=t3l0, 2=t3l1, 3=t1, 4=t0l0, 5=t0l1
    K = NL * D
    w_all = np.zeros((K, 6 * P), np.float32)
    w_all[:, W_T2 * P:(W_T2 + 1) * P] = w_of(2)
    w_all[:, W_T1 * P:(W_T1 + 1) * P] = w_of(1)
    for li in range(2):
        lmask = (l_of == li).astype(np.float32)[None, :]
        w_all[:, (W_T0L0 + li) * P:(W_T0L0 + li + 1) * P] = w_of(0) * lmask
        w_all[:, (W_T3L0 + li) * P:(W_T3L0 + li + 1) * P] = w_of(3) * lmask
    bias_q = np.ascontiguousarray(
        bias.reshape(S)[s_of].astype(np.float32).reshape(P, 1))

    w_all = np.ascontiguousarray(w_all.astype(ml_dtypes.bfloat16))
    in_maps = []
    for core in range(N_CORES):
        x2 = np.ascontiguousarray(
            x[NL * core:NL * (core + 1)].reshape(NL * D, M)
            .astype(ml_dtypes.bfloat16))
        in_maps.append({"x2": x2, "w_all": w_all, "bias_q": bias_q})
    return in_maps


def run(x, coefs, bias, **run_kwargs):
    """Run on hardware; returns (full_output, BassKernelResults)."""
    from concourse.bass_utils import run_bass_kernel_spmd

    prog = _get_prog()
    in_maps = _make_in_maps(x, coefs, bias)
    res = run_bass_kernel_spmd(prog, in_maps, list(range(N_CORES)), **run_kwargs)
    # y rows are (r, l, ns)-major (row p = r*128 + q, q = l*64 + ns holds
    # i = 24*l + r): un-permute on the host to (n, s, i, j, k)
    out = np.concatenate(
        [res.results[i]["y"].reshape(HALF, 2, NL, S, M, M)
         .transpose(2, 3, 1, 0, 4, 5).reshape(NL, S, M, M, M)
         for i in range(N_CORES)],
        axis=0).astype(np.float32)
    return out, res


def kernel(x, coefs, bias):
    out, _ = run(x, coefs, bias)
    return out


# revision 31
# speedup vs baseline: 1.0754x; 1.0754x over previous
"""Trainium2 Bass kernel for nn_Eq1to3 (eset_ops_1_to_3 + einsum broadcast expansion).

Reference computation (N=16, D=64, S=32, M=48, BASIS=4):
    t[b,n,s,m] = sum_d coefs[d,s,b] * x[n,d,m]        # tiny einsum
    out[n,s,i,j,k] = t0[n,s,i] + t1[n,s,j] + t2[n,s,k]
                     + (i==j==k) * t3[n,s,i] + bias[s]
Output (16, 32, 48, 48, 48) f32 = 226.5 MB. The correctness gate is
scale-relative 2e-2, so the HW computes in bf16 (inputs rounded to bf16,
fp32 PSUM accumulation) and writes bf16 (113 MB total, 14.16 MB per core
=> ~40 us HBM floor at ~358 GB/s); the host upcasts to f32.

Strategy: data-parallel over N across 8 cores (2 batches/core). Per core the
output is [3072 rows p=(n,s,i), 2304 cols (j,k)] in bf16.

Row->partition assignment: partition q = l*64 + ns holds rows
i(q,r) = 24*l + r of (n,s)-group ns = q % 64, l = q // 64, r in [0,24).
The l-halves are CONTIGUOUS partition ranges, so the superdiagonal add
needs no mask tensor: for each half, the diagonal of row u sits at flat
offset u*2304 + i*49 with i = 24*l + r0 + u, i.e. one strided AP of
stride 2353. Every per-partition table is a gather the TENSOR ENGINE
produces directly with host-prepared indicator weights, folding the batch
index into the contraction: lhsT[(n',d), q] = coefs[d, s(q), b]*(n'==n(q)),
against rhs built from x2[(n,d), m] = x[n,d,m] (bf16 matmuls, fp32 PSUM):

    B_ps[q, (r,j)]  = t0[ns(q), i(q,r)] + t1[ns(q), j]
    T2_ps[q, k]     = t2[ns(q), k]
    T3_ps[q, r]     = t3[ns(q), i(q,r)]

ACT mirrors each PSUM B chunk into SBUF bf16 with every value DUPLICATED in
adjacent pairs, B2[q, (r, j, 2)], and T2 (+bias via the ACT per-partition
bias operand) is expanded once into T2E[q, (j, k)] = t2[k] + bias[s] by a
DVE tensor_copy (packed APs -> 4x perf mode). The main loop runs one DVE
tensor_tensor per output row with loop dims (j, 24, 2):

    out[q, (j, t, p)] = B2[q, (r, j, p)](t-bcast) + T2E[q, (j, t, p)]

All three APs are 2-byte, SBUF-resident, with a packed innermost [1,2] dim,
which makes the op eligible for the DVE 2x 16-bit perf mode (~1.35 us/row;
a stride-0 innermost broadcast would pin it at 1x). The t3 superdiagonal
lands as two tiny strided DVE adds per group (one per l-half) read straight
from T3G, so gpsimd carries no compute at all. Output groups stream over
three DMA paths - SP HWDGE, ACT HWDGE (5.31 MB each) and gpsimd SWDGE
(3.54 MB, mid-stream groups) - with 4 single-row groups first (early
stream start), 8 two-row groups, then 4 single-row groups, the last two on
the HWDGE rings so everything drains together. Startup is gated by the
weight load, so w_all is split into two DMAs with the t2/t3 blocks first
(they gate T2E and the first diag), and B2 chunk copies are interleaved
into the ACT queue behind DMA triggers they cannot block.
"""

import numpy as np
import ml_dtypes

N, D, S, M, BASIS = 16, 64, 32, 48, 4
N_CORES = 8
NL = N // N_CORES              # batches per core (2)
NS = NL * S                    # (n,s) groups per core (64)
ROWS = NS * M                  # output rows per core (3072)
JK = M * M                     # free size per row (2304)
P = 128                        # partitions
HALF = M // 2                  # rows per partition (24)
# B-matmul chunk sizes in i'-rows; small first chunks let the HBM write
# stream start before the rest of B exists
CHUNKS = [2, 2, 4, 8, 8]
CHUNK_R0 = [sum(CHUNKS[:c]) for c in range(len(CHUNKS))]
# w_all column blocks (order chosen so the first DMA part carries what
# gates the start of the stream: t2 -> T2E, t3 -> the first diag adds);
# bias is folded into T2G via the ACT per-partition bias operand
W_T2, W_T3L0, W_T3L1, W_T1, W_T0L0, W_T0L1 = range(6)
W_PARTS = [3, 3]               # DMA split: blocks [0,3) then [3,6)

_PROG = None


def _build_prog():
    import concourse.bacc as bacc
    import concourse.tile as tile
    import concourse.mybir as mybir

    bf16 = mybir.dt.bfloat16
    copy_fn = mybir.ActivationFunctionType.Copy
    nc = bacc.Bacc("TRN2", target_bir_lowering=False, debug=False,
                   num_devices=N_CORES)

    w_all_d = nc.dram_tensor("w_all", [NL * D, 6 * P], bf16,
                             kind="ExternalInput").ap()
    bias_d = nc.dram_tensor("bias_q", [P, 1], mybir.dt.float32,
                            kind="ExternalInput").ap()
    x2_d = nc.dram_tensor("x2", [NL * D, M], bf16, kind="ExternalInput").ap()
    y_d = nc.dram_tensor("y", [ROWS, JK], bf16, kind="ExternalOutput").ap()

    K = NL * D                  # contraction size (128)

    with tile.TileContext(nc) as tc:
        with (
            tc.tile_pool(name="const", bufs=1) as cpool,
            tc.tile_pool(name="psum", bufs=1, space="PSUM") as ppool,
            tc.tile_pool(name="outp", bufs=6) as opool,
        ):
            # ---- load inputs; sync ring: x2 + bias, scalar ring: w parts
            x2_sb = cpool.tile([K, M], bf16)
            nc.sync.dma_start(out=x2_sb[:], in_=x2_d[:])
            bias_sb = cpool.tile([P, 1], mybir.dt.float32)
            nc.sync.dma_start(out=bias_sb[:], in_=bias_d[:])
            w_sb = cpool.tile([K, 6 * P], bf16)
            lo = 0
            for nblk in W_PARTS:
                nc.scalar.dma_start(out=w_sb[:, lo * P:(lo + nblk) * P],
                                    in_=w_all_d[:, lo * P:(lo + nblk) * P])
                lo += nblk

            def w_blk(idx):
                return w_sb[:, idx * P:(idx + 1) * P]

            # ---- T2 path first: it gates T2E which gates every row op ----
            T2_ps = ppool.tile([P, M], mybir.dt.float32)
            nc.tensor.matmul(T2_ps[:], w_blk(W_T2), x2_sb[:],
                             start=True, stop=True)
            # ---- T3: gates the diag adds ----
            T3_ps = ppool.tile([P, HALF], mybir.dt.float32)
            for li in range(2):
                nc.tensor.matmul(T3_ps[:], w_blk(W_T3L0 + li),
                                 x2_sb[:, HALF * li:HALF * (li + 1)],
                                 start=(li == 0), stop=(li == 1))

            # bias[s(q)] rides in here: every output element sees T2E once
            T2G = cpool.tile([P, M], bf16)
            nc.scalar.activation(T2G[:], T2_ps[:],
                                 mybir.ActivationFunctionType.Identity,
                                 bias=bias_sb[:, 0:1])
            T3G = cpool.tile([P, HALF], bf16)
            nc.scalar.activation(T3G[:], T3_ps[:], copy_fn)
            HP = P // 2                # partitions per l-half (64)

            def build_t2e(with_diag_of_row):
                # T2E[q, (j, k)] = t2[k]: DVE tensor_copy, all APs packed
                # bf16 SBUF (4x perf mode) - the j-broadcast is an outer
                # dim. Optionally folds row r's t3 diag element in, making
                # that row a single DVE op with no post-diag pass.
                t = cpool.tile([P, JK], bf16)
                nc.vector.tensor_copy(
                    out=t.rearrange("q (j k) -> q j k", k=M),
                    in_=T2G[:, None, :].broadcast_to((P, M, M)))
                if with_diag_of_row is not None:
                    r = with_diag_of_row
                    for l in range(2):
                        s0 = (HALF * l + r) * (M + 1)
                        dv = t[l * HP:(l + 1) * HP, s0:s0 + 1]
                        tg = T3G[l * HP:(l + 1) * HP, r:r + 1]
                        nc.vector.tensor_add(out=dv, in0=dv, in1=tg)
                return t

            # ---- B[q, (r, j)] via accumulating bf16 matmuls, one
            # bank-aligned PSUM tile per chunk (a PE-write and an ACT-read
            # in the same PSUM bank is a hardware fault) ----
            B_chunks = [ppool.tile([P, ci * M], mybir.dt.float32,
                                   name=f"B_ps{c}")
                        for c, ci in enumerate(CHUNKS)]

            def emit_b_chunk(c):
                ci = CHUNKS[c]
                i0 = CHUNK_R0[c]
                blk = B_chunks[c].rearrange("q (r j) -> q r j", j=M)
                # t1 part: rhs[(n'd), (r, j)] = x[n', d, j]
                rhs = x2_sb[:, None, :].broadcast_to((K, ci, M))
                nc.tensor.matmul(blk, w_blk(W_T1), rhs,
                                 start=True, stop=False)
                for li in range(2):
                    # t0 part: rhs[(n'd), (r, j)] = x[n', d, 24*li + i0 + r]
                    rhs = x2_sb[:, HALF * li + i0:HALF * li + i0 + ci]
                    rhs = rhs[:, :, None].broadcast_to((K, ci, M))
                    nc.tensor.matmul(blk, w_blk(W_T0L0 + li), rhs,
                                     start=False, stop=(li == 1))

            # SBUF bf16 mirror of B with values duplicated in pairs:
            # B2[q, (r, j, p)] = B[q, (r, j)] for p=0,1
            B2 = cpool.tile([P, HALF * M * 2], bf16)
            B2v = B2.rearrange("q (r j p) -> q r j p", j=M, p=2)

            def copy_b2(c):
                ci = CHUNKS[c]
                i0 = CHUNK_R0[c]
                src = B_chunks[c].rearrange("q (r j) -> q r j", j=M)
                src = src[:, :, :, None].broadcast_to((P, ci, M, 2))
                nc.scalar.activation(B2v[:, i0:i0 + ci], src, copy_fn)

            # ---- main loop over row-slices r: y row p = r*128 + q, so a
            # group DMA writes one dense contiguous HBM region (partition
            # stride = one 4608 B row) instead of 128 scattered blocks ----
            y_v = y_d.rearrange("(r q) f -> q r f", r=HALF)

            def emit_group(r0, rw, ring, t2e, post_diag=True):
                out_t = opool.tile([P, rw * JK], bf16, tag="out")
                o5 = out_t.rearrange("q (u j t p) -> q u j t p",
                                     u=rw, j=M, p=2)
                t2e5 = t2e.rearrange("q (j t p) -> q j t p", j=M, p=2)
                for u in range(rw):
                    in0 = B2v[:, r0 + u, :, None, :]
                    in0 = in0.broadcast_to((P, M, HALF, 2))
                    nc.vector.tensor_add(out=o5[:, u], in0=in0, in1=t2e5)
                # superdiagonal of row u at flat offset u*2304 + i*49,
                # i = 24*l + r0 + u with l = q//64: stride-2353 AP over
                # each contiguous partition half, straight from T3G - two
                # tiny DVE adds, no mask tensor (the host un-permutes the
                # l-major row order during the gather)
                if post_diag:
                    for l in range(2):
                        half = out_t[l * HP:(l + 1) * HP]
                        s0 = (HALF * l + r0) * (M + 1)
                        dv = half[:, s0::JK + M + 1][:, :rw]
                        tg = T3G[l * HP:(l + 1) * HP, r0:r0 + rw]
                        nc.vector.tensor_add(out=dv, in0=dv, in1=tg)
                ring.dma_start(
                    out=y_v[:, r0:r0 + rw, :],
                    in_=out_t.rearrange("q (u f) -> q u f", u=rw))

            # Row 0 streams via T2E0 (diag pre-folded, built in DVE's idle
            # startup window) so the first DMA fires straight after one
            # row op; everything else uses the shared T2E. Later B2 copies
            # are interleaved so they sit behind DMA triggers they cannot
            # delay. Output rings: SP HWDGE, ACT HWDGE and gpsimd SWDGE
            # carry 4.72 MB each; the last singles end on the two HWDGE
            # rings so everything drains together.
            S, A, G = nc.sync, nc.scalar, nc.gpsimd
            T2E0 = build_t2e(with_diag_of_row=0)
            emit_b_chunk(0)
            copy_b2(0)
            emit_group(0, 1, S, T2E0, post_diag=False)
            T2E = build_t2e(with_diag_of_row=None)
            emit_b_chunk(1)
            copy_b2(1)
            emit_group(1, 1, A, T2E)
            emit_b_chunk(2)
            emit_group(2, 1, S, T2E)
            emit_group(3, 1, A, T2E)
            copy_b2(2)
            emit_b_chunk(3)
            emit_group(4, 2, S, T2E)
            emit_group(6, 2, A, T2E)
            copy_b2(3)
            emit_b_chunk(4)
            emit_group(8, 2, G, T2E)
            emit_group(10, 2, S, T2E)
            copy_b2(4)
            emit_group(12, 2, A, T2E)
            emit_group(14, 2, G, T2E)
            emit_group(16, 2, S, T2E)
            emit_group(18, 2, A, T2E)
            emit_group(20, 1, G, T2E)
            emit_group(21, 1, G, T2E)
            emit_group(22, 1, A, T2E)
            emit_group(23, 1, S, T2E)

    nc.compile()
    return nc


def _get_prog():
    global _PROG
    if _PROG is None:
        _PROG = _build_prog()
    return _PROG


def _make_in_maps(x, coefs, bias):
    x = np.asarray(x, dtype=np.float32)
    coefs = np.asarray(coefs, dtype=np.float32)
    bias = np.asarray(bias, dtype=np.float32)

    # partition q = l*64 + ns: ns(q) = q % 64 = n*32 + s, l(q) = q // 64
    q = np.arange(P)
    ns_of = q % NS
    n_of = ns_of // S
    s_of = ns_of % S
    l_of = q // NS
    # indicator weights w_b[(n',d), q] = coefs[d, s(q), b] * (n' == n(q))
    nd_n = np.repeat(np.arange(NL), D)                # (K,) n' of row
    nd_d = np.tile(np.arange(D), NL)                  # (K,) d of row
    sel = (nd_n[:, None] == n_of[None, :]).astype(np.float32)  # (K, P)

    def w_of(b):
        return coefs[nd_d[:, None], s_of[None, :], b] * sel

    # column blocks: 0=t2, 1=t3l0, 2=t3l1, 3=t1, 4=t0l0, 5=t0l1
    K = NL * D
    w_all = np.zeros((K, 6 * P), np.float32)
    w_all[:, W_T2 * P:(W_T2 + 1) * P] = w_of(2)
    w_all[:, W_T1 * P:(W_T1 + 1) * P] = w_of(1)
    for li in range(2):
        lmask = (l_of == li).astype(np.float32)[None, :]
        w_all[:, (W_T0L0 + li) * P:(W_T0L0 + li + 1) * P] = w_of(0) * lmask
        w_all[:, (W_T3L0 + li) * P:(W_T3L0 + li + 1) * P] = w_of(3) * lmask
    bias_q = np.ascontiguousarray(
        bias.reshape(S)[s_of].astype(np.float32).reshape(P, 1))

    w_all = np.ascontiguousarray(w_all.astype(ml_dtypes.bfloat16))
    in_maps = []
    for core in range(N_CORES):
        x2 = np.ascontiguousarray(
            x[NL * core:NL * (core + 1)].reshape(NL * D, M)
            .astype(ml_dtypes.bfloat16))
        in_maps.append({"x2": x2, "w_all": w_all, "bias_q": bias_q})
    return in_maps


def run(x, coefs, bias, **run_kwargs):
    """Run on hardware; returns (full_output, BassKernelResults)."""
    from concourse.bass_utils import run_bass_kernel_spmd

    prog = _get_prog()
    in_maps = _make_in_maps(x, coefs, bias)
    res = run_bass_kernel_spmd(prog, in_maps, list(range(N_CORES)), **run_kwargs)
    # y rows are (r, l, ns)-major (row p = r*128 + q, q = l*64 + ns holds
    # i = 24*l + r): un-permute on the host to (n, s, i, j, k)
    out = np.concatenate(
        [res.results[i]["y"].reshape(HALF, 2, NL, S, M, M)
         .transpose(2, 3, 1, 0, 4, 5).reshape(NL, S, M, M, M)
         for i in range(N_CORES)],
        axis=0).astype(np.float32)
    return out, res


def kernel(x, coefs, bias):
    out, _ = run(x, coefs, bias)
    return out
